# revision 15
# baseline (speedup 1.0000x reference)
"""MoE expert FFN kernel for Trainium2 (8 NeuronCores, expert-parallel).

Problem: 8 experts, each with 1024 routed tokens:
    gate_up = x_e @ Wgu_e        # [1024,2048] @ [2048,12288]
    hidden  = silu(gate) * up    # [1024,6144]
    out_e   = hidden @ Wd_e      # [1024,6144] @ [6144,2048]

Sharding: expert-parallel, one expert per core, no collectives.

Per-core kernel (everything transposed so the contraction dim sits on
SBUF partitions):
  Phase A: gate_up^T tiles [128f x 1024t] = sum_k Wgu[k-block, f-block].T @ x^T[k-block, :]
           bf16 matmuls, fp32 PSUM; silu on ScalarE, gating mul on VectorE,
           hidden^T kept resident in SBUF as bf16 [128, 48, 1024].
  Phase B: out^T tiles [128d x 1024t] = sum_j Wd[i-block j, d-block].T @ hidden^T[i-block j, :]

Host side: shards tokens/weights per expert, pre-transposes weight tiles into
DMA-friendly layouts, casts to bf16, and transposes outputs back.
"""

import json
import os

import numpy as np
import ml_dtypes

import concourse.mybir as mybir
import concourse.tile as tile
from concourse import bacc, bass_utils

E = 8            # experts == cores
T = 1024         # tokens per expert
D = 2048         # hidden
I = 6144         # intermediate
P = 128
KT = D // P      # 16 k-tiles over hidden dim
FT = 2 * I // P  # 96 f-tiles over gate+up dim
JT = I // P      # 48 i-tiles over intermediate dim
DT = D // P      # 16 d-tiles over output dim
TH = T // 2      # 512, PSUM bank free-dim
WDC = 8          # wd DMA chunk: i-tiles per transfer

BF16 = mybir.dt.bfloat16
F32 = mybir.dt.float32

_CACHE = {}


def _ldw_key(inst):
    d = json.loads(mybir.instruction_to_pretty_json_string(inst))
    d.pop("name", None)
    d.pop("sync_info", None)
    return json.dumps(d, sort_keys=True)


def _dedup_ldweights(nc):
    """Drop InstLdweights that reload the weights already resident in the PE
    array. tile_legalize emits one LDWEIGHTS per matmul; our matmul pairs
    (two 512-token halves per weight tile) make half of them redundant.
    Only sync-free LDWs whose AP matches the previous load with nothing but
    matmuls in between are dropped, so all semaphore protocols are intact.
    Cuts the PE instruction stream ~25%."""
    PE = mybir.EngineType.PE
    dropped = 0
    for blk in nc.main_func.blocks:
        keep = []
        last_key = None  # weights currently in the PE array
        for inst in blk.instructions:
            if getattr(inst, "engine", None) == PE:
                if isinstance(inst, mybir.InstLdweights):
                    si = inst.sync_info
                    clean = si is None or (not si.on_wait and not si.on_update)
                    k = _ldw_key(inst)
                    if clean and k == last_key:
                        dropped += 1
                        continue
                    last_key = k
                elif isinstance(inst, mybir.InstMatmult):
                    pass  # reads weights, doesn't clobber them
                else:
                    last_key = None
            keep.append(inst)
        blk.instructions[:] = keep
    return dropped


def _build():
    nc = bacc.Bacc("TRN2", target_bir_lowering=False, debug=False, num_devices=E)
    xt = nc.dram_tensor("xt", [D, T], BF16, kind="ExternalInput").ap()
    wgu = nc.dram_tensor("wgu", [FT, P, D], BF16, kind="ExternalInput").ap()
    # wd: [d-tile, j-chunk of 8 i-tiles, p, 8*128] so each DMA moves 2KB/partition
    wd = nc.dram_tensor(
        "wd", [DT, JT // WDC, P, WDC * P], BF16, kind="ExternalInput"
    ).ap()
    outt = nc.dram_tensor("outt", [D, T], F32, kind="ExternalOutput").ap()

    with tile.TileContext(nc) as tc:
        with (
            tc.tile_pool(name="xpool", bufs=1) as xpool,
            tc.tile_pool(name="hpool", bufs=1) as hpool,
            tc.tile_pool(name="wg", bufs=4) as wgpool,
            tc.tile_pool(name="wdp", bufs=8) as wdpool,
            tc.tile_pool(name="act", bufs=4) as actpool,
            tc.tile_pool(name="opool", bufs=3) as opool,
            tc.tile_pool(name="ps", bufs=8, space="PSUM") as ps,
        ):
            # Warmup matmuls on dummy data: run while the first DMAs are in
            # flight so the PE's HAM clock-gate is already at full rate when
            # real work arrives (~3.4us of sustained PE activity required).
            warm_w = wgpool.tile([P, P], BF16, tag="warmw")
            nc.any.memset(warm_w[:], 0.0)
            warm_x = wgpool.tile([P, TH], BF16, tag="warmx")
            nc.any.memset(warm_x[:], 0.0)
            warm_ps = ps.tile([P, TH], F32, tag="ps")
            for _ in range(14):
                nc.tensor.matmul(warm_ps[:], warm_w[:], warm_x[:], start=True, stop=True)

            # First j-iteration's weights before xt so the PE can start ASAP.
            wg0 = wgpool.tile([P, D], BF16, tag="w")
            nc.sync.dma_start(wg0[:], wgu[0])
            wu0 = wgpool.tile([P, D], BF16, tag="w")
            nc.sync.dma_start(wu0[:], wgu[JT])

            # x^T resident in SBUF: per-k-slice tiles so MM k only waits for
            # slice k. [128, 1024 tokens] bf16 each.
            xt_r = xt.rearrange("(k p) t -> p k t", p=P)
            xt_sb = []
            for k in range(KT):
                xk = xpool.tile([P, T], BF16, tag=f"x{k}")
                nc.sync.dma_start(xk[:], xt_r[:, k, :])
                xt_sb.append(xk)

            # hidden^T resident in SBUF: [128, 48 i-tiles, 1024 tokens] bf16
            hid_sb = hpool.tile([P, JT, T], BF16)

            # ---- Phase A: gate_up matmul + silu*up ----
            for j in range(JT):
                if j == 0:
                    wg, wu = wg0, wu0
                else:
                    wg = wgpool.tile([P, D], BF16, tag="w")
                    nc.sync.dma_start(wg[:], wgu[j])
                    wu = wgpool.tile([P, D], BF16, tag="w")
                    nc.sync.dma_start(wu[:], wgu[j + JT])

                pg0 = ps.tile([P, TH], F32, tag="ps")
                pg1 = ps.tile([P, TH], F32, tag="ps")
                pu0 = ps.tile([P, TH], F32, tag="ps")
                pu1 = ps.tile([P, TH], F32, tag="ps")
                for k in range(KT):
                    st, sp = k == 0, k == KT - 1
                    wgk = wg[:, k * P:(k + 1) * P]
                    wuk = wu[:, k * P:(k + 1) * P]
                    xk = xt_sb[k]
                    nc.tensor.matmul(pg0[:], wgk, xk[:, :TH], start=st, stop=sp)
                    nc.tensor.matmul(pg1[:], wgk, xk[:, TH:], start=st, stop=sp)
                    nc.tensor.matmul(pu0[:], wuk, xk[:, :TH], start=st, stop=sp)
                    nc.tensor.matmul(pu1[:], wuk, xk[:, TH:], start=st, stop=sp)

                for h, (pg, pu) in enumerate(((pg0, pu0), (pg1, pu1))):
                    s = actpool.tile([P, TH], F32, tag="silu")
                    nc.scalar.activation(s[:], pg[:], mybir.ActivationFunctionType.Silu)
                    nc.vector.tensor_mul(
                        out=hid_sb[:, j, h * TH:(h + 1) * TH], in0=s[:], in1=pu[:]
                    )

            # ---- Phase B: down-projection ----
            for t2 in range(DT):
                po0 = ps.tile([P, TH], F32, tag="ps")
                po1 = ps.tile([P, TH], F32, tag="ps")
                for jc in range(JT // WDC):
                    wt = wdpool.tile([P, WDC * P], BF16, tag="wd")
                    nc.sync.dma_start(wt[:], wd[t2, jc])
                    for jj in range(WDC):
                        j = jc * WDC + jj
                        st, sp = j == 0, j == JT - 1
                        wtj = wt[:, jj * P:(jj + 1) * P]
                        nc.tensor.matmul(
                            po0[:], wtj, hid_sb[:, j, :TH], start=st, stop=sp
                        )
                        nc.tensor.matmul(
                            po1[:], wtj, hid_sb[:, j, TH:], start=st, stop=sp
                        )
                ob = opool.tile([P, T], F32, tag="out")
                nc.vector.tensor_copy(ob[:, :TH], po0[:])
                nc.sync.dma_start(outt[t2 * P:(t2 + 1) * P, :TH], ob[:, :TH])
                nc.vector.tensor_copy(ob[:, TH:], po1[:])
                nc.sync.dma_start(outt[t2 * P:(t2 + 1) * P, TH:], ob[:, TH:])

    _dedup_ldweights(nc)
    nc.compile()
    return nc


def _prep_inputs(routed_tokens, w_gate_up, w_down):
    """Shard per expert + pre-arrange into the kernel's DMA layouts (bf16)."""
    bf = ml_dtypes.bfloat16
    routed_tokens = np.asarray(routed_tokens, dtype=np.float32)
    w_gate_up = np.asarray(w_gate_up, dtype=np.float32)
    w_down = np.asarray(w_down, dtype=np.float32)
    x = np.ascontiguousarray(routed_tokens.reshape(E, T, D))
    in_maps = []
    for e in range(E):
        xt_e = np.ascontiguousarray(x[e].T).astype(bf)  # [D, T]
        # Wgu[d, f] -> [f-tile j, p(=d within block), k-tile*128 + fc]
        wgu_e = (
            w_gate_up[e]
            .reshape(KT, P, FT, P)
            .transpose(2, 1, 0, 3)
            .reshape(FT, P, D)
            .astype(bf)
        )
        # Wd[i, d] -> [d-tile t2, j-chunk, p(=i within block), jj*128 + dc]
        wd_e = (
            w_down[e]
            .reshape(JT // WDC, WDC, P, DT, P)
            .transpose(3, 0, 2, 1, 4)
            .reshape(DT, JT // WDC, P, WDC * P)
            .astype(bf)
        )
        in_maps.append(
            {
                "xt": xt_e,
                "wgu": np.ascontiguousarray(wgu_e),
                "wd": np.ascontiguousarray(wd_e),
            }
        )
    return in_maps


LAST_RESULTS = None


def kernel(routed_tokens, w_gate_up, w_down):
    global LAST_RESULTS
    if "nc" not in _CACHE:
        _CACHE["nc"] = _build()
    nc = _CACHE["nc"]

    in_maps = _prep_inputs(routed_tokens, w_gate_up, w_down)
    try:
        res = bass_utils.run_bass_kernel_spmd(nc, in_maps, core_ids=list(range(E)))
    except ModuleNotFoundError:
        # BASS_TRACE set but the axon NTFF hook isn't importable here —
        # retry with tracing hard-disabled.
        os.environ["BASS_NEVER_TRACE"] = "1"
        res = bass_utils.run_bass_kernel_spmd(nc, in_maps, core_ids=list(range(E)))
    LAST_RESULTS = res

    out = np.empty((E, T, D), dtype=np.float32)
    for e in range(E):
        out[e] = res.results[e]["outt"].T
    return out.reshape(E * T, D)


# revision 20
# speedup vs baseline: 1.0728x; 1.0728x over previous
"""MoE expert FFN kernel for Trainium2 (8 NeuronCores, expert-parallel).

Problem: 8 experts, each with 1024 routed tokens:
    gate_up = x_e @ Wgu_e        # [1024,2048] @ [2048,12288]
    hidden  = silu(gate) * up    # [1024,6144]
    out_e   = hidden @ Wd_e      # [1024,6144] @ [6144,2048]

Sharding: expert-parallel, one expert per core, no collectives.

Per-core kernel (everything transposed so the contraction dim sits on
SBUF partitions):
  Phase A: gate_up^T tiles [128f x 1024t] = sum_k Wgu[k-block, f-block].T @ x^T[k-block, :]
           bf16 matmuls, fp32 PSUM; silu on ScalarE, gating mul on VectorE,
           hidden^T kept resident in SBUF as bf16 [128, 48, 1024].
  Phase B: out^T tiles [128d x 1024t] = sum_j Wd[i-block j, d-block].T @ hidden^T[i-block j, :]

Host side: shards tokens/weights per expert, pre-transposes weight tiles into
DMA-friendly layouts, casts to bf16, and transposes outputs back.
"""

import os

import numpy as np
import ml_dtypes

import concourse.mybir as mybir
import concourse.tile as tile
from concourse import bacc, bass_utils

E = 8            # experts == cores
T = 1024         # tokens per expert
D = 2048         # hidden
I = 6144         # intermediate
P = 128
KT = D // P      # 16 k-tiles over hidden dim
FT = 2 * I // P  # 96 f-tiles over gate+up dim
JT = I // P      # 48 i-tiles over intermediate dim
DT = D // P      # 16 d-tiles over output dim
TH = T // 2      # 512, PSUM bank free-dim
WDC = 8          # wd DMA chunk: i-tiles per transfer

BF16 = mybir.dt.bfloat16
F32 = mybir.dt.float32

_CACHE = {}


def _build():
    nc = bacc.Bacc("TRN2", target_bir_lowering=False, debug=False, num_devices=E)
    xt = nc.dram_tensor("xt", [D, T], BF16, kind="ExternalInput").ap()
    wgu = nc.dram_tensor("wgu", [FT, P, D], BF16, kind="ExternalInput").ap()
    # wd: [d-tile, j-chunk of 8 i-tiles, p, 8*128] so each DMA moves 2KB/partition
    wd = nc.dram_tensor(
        "wd", [DT, JT // WDC, P, WDC * P], BF16, kind="ExternalInput"
    ).ap()
    outt = nc.dram_tensor("outt", [D, T], F32, kind="ExternalOutput").ap()

    with tile.TileContext(nc) as tc:
        with (
            tc.tile_pool(name="xpool", bufs=1) as xpool,
            tc.tile_pool(name="hpool", bufs=1) as hpool,
            tc.tile_pool(name="wg", bufs=4) as wgpool,
            tc.tile_pool(name="wdp", bufs=8) as wdpool,
            tc.tile_pool(name="act", bufs=4) as actpool,
            tc.tile_pool(name="opool", bufs=3) as opool,
            tc.tile_pool(name="ps", bufs=8, space="PSUM") as ps,
        ):
            # Warmup matmuls on dummy data: run while the first DMAs are in
            # flight so the PE's HAM clock-gate is already at full rate when
            # real work arrives (~3.4us of sustained PE activity required).
            # memset on VectorE (not nc.any -> GpSimd) so the GpSimd engine
            # has no instructions at all.
            warm_w = wgpool.tile([P, P], BF16, tag="warmw")
            nc.vector.memset(warm_w[:], 0.0)
            warm_x = wgpool.tile([P, TH], BF16, tag="warmx")
            nc.vector.memset(warm_x[:], 0.0)
            warm_ps = ps.tile([P, TH], F32, tag="ps")
            for _ in range(14):
                nc.tensor.matmul(warm_ps[:], warm_w[:], warm_x[:], start=True, stop=True)

            # First j-iteration's weights before xt so the PE can start ASAP;
            # split in halves so transfers spread over more DMA queues.
            wg0 = wgpool.tile([P, D], BF16, tag="w")
            nc.sync.dma_start(wg0[:, :D // 2], wgu[0][:, :D // 2])
            nc.sync.dma_start(wg0[:, D // 2:], wgu[0][:, D // 2:])
            wu0 = wgpool.tile([P, D], BF16, tag="w")
            nc.sync.dma_start(wu0[:, :D // 2], wgu[JT][:, :D // 2])
            nc.sync.dma_start(wu0[:, D // 2:], wgu[JT][:, D // 2:])

            # x^T resident in SBUF: per-k-slice tiles so MM k only waits for
            # slice k. [128, 1024 tokens] bf16 each.
            xt_r = xt.rearrange("(k p) t -> p k t", p=P)
            xt_sb = []
            for k in range(KT):
                xk = xpool.tile([P, T], BF16, tag=f"x{k}")
                nc.sync.dma_start(xk[:], xt_r[:, k, :])
                xt_sb.append(xk)

            # hidden^T resident in SBUF: [128, 48 i-tiles, 1024 tokens] bf16
            hid_sb = hpool.tile([P, JT, T], BF16)

            # ---- Phase A: gate_up matmul + silu*up ----
            for j in range(JT):
                if j == 0:
                    wg, wu = wg0, wu0
                else:
                    wg = wgpool.tile([P, D], BF16, tag="w")
                    nc.sync.dma_start(wg[:], wgu[j])
                    wu = wgpool.tile([P, D], BF16, tag="w")
                    nc.sync.dma_start(wu[:], wgu[j + JT])

                pg0 = ps.tile([P, TH], F32, tag="ps")
                pg1 = ps.tile([P, TH], F32, tag="ps")
                pu0 = ps.tile([P, TH], F32, tag="ps")
                pu1 = ps.tile([P, TH], F32, tag="ps")
                for k in range(KT):
                    st, sp = k == 0, k == KT - 1
                    wgk = wg[:, k * P:(k + 1) * P]
                    wuk = wu[:, k * P:(k + 1) * P]
                    xk = xt_sb[k]
                    nc.tensor.matmul(pg0[:], wgk, xk[:, :TH], start=st, stop=sp)
                    nc.tensor.matmul(pg1[:], wgk, xk[:, TH:], start=st, stop=sp)
                    nc.tensor.matmul(pu0[:], wuk, xk[:, :TH], start=st, stop=sp)
                    nc.tensor.matmul(pu1[:], wuk, xk[:, TH:], start=st, stop=sp)

                for h, (pg, pu) in enumerate(((pg0, pu0), (pg1, pu1))):
                    s = actpool.tile([P, TH], F32, tag="silu")
                    nc.scalar.activation(s[:], pg[:], mybir.ActivationFunctionType.Silu)
                    nc.vector.tensor_mul(
                        out=hid_sb[:, j, h * TH:(h + 1) * TH], in0=s[:], in1=pu[:]
                    )

            # ---- Phase B: down-projection ----
            for t2 in range(DT):
                po0 = ps.tile([P, TH], F32, tag="ps")
                po1 = ps.tile([P, TH], F32, tag="ps")
                for jc in range(JT // WDC):
                    wt = wdpool.tile([P, WDC * P], BF16, tag="wd")
                    nc.sync.dma_start(wt[:], wd[t2, jc])
                    for jj in range(WDC):
                        j = jc * WDC + jj
                        st, sp = j == 0, j == JT - 1
                        wtj = wt[:, jj * P:(jj + 1) * P]
                        nc.tensor.matmul(
                            po0[:], wtj, hid_sb[:, j, :TH], start=st, stop=sp
                        )
                        nc.tensor.matmul(
                            po1[:], wtj, hid_sb[:, j, TH:], start=st, stop=sp
                        )
                ob = opool.tile([P, T], F32, tag="out")
                nc.vector.tensor_copy(ob[:, :TH], po0[:])
                nc.sync.dma_start(outt[t2 * P:(t2 + 1) * P, :TH], ob[:, :TH])
                nc.vector.tensor_copy(ob[:, TH:], po1[:])
                nc.sync.dma_start(outt[t2 * P:(t2 + 1) * P, TH:], ob[:, TH:])

    # NOTE: an LDW-dedup pass (drop the second LDWEIGHTS of each matmul
    # pair) was tried and REVERTED: removing the redundant loads slows the
    # second matmul of each pair from ~215ns to ~256ns — the extra LDW is
    # free (hidden) and appears to enable fill/drain overlap between
    # back-to-back matmuls. Net -75us. Keep one LDW per matmul.
    nc.compile()
    return nc


def _prep_inputs(routed_tokens, w_gate_up, w_down):
    """Shard per expert + pre-arrange into the kernel's DMA layouts (bf16)."""
    bf = ml_dtypes.bfloat16
    routed_tokens = np.asarray(routed_tokens, dtype=np.float32)
    w_gate_up = np.asarray(w_gate_up, dtype=np.float32)
    w_down = np.asarray(w_down, dtype=np.float32)
    x = np.ascontiguousarray(routed_tokens.reshape(E, T, D))
    in_maps = []
    for e in range(E):
        xt_e = np.ascontiguousarray(x[e].T).astype(bf)  # [D, T]
        # Wgu[d, f] -> [f-tile j, p(=d within block), k-tile*128 + fc]
        wgu_e = (
            w_gate_up[e]
            .reshape(KT, P, FT, P)
            .transpose(2, 1, 0, 3)
            .reshape(FT, P, D)
            .astype(bf)
        )
        # Wd[i, d] -> [d-tile t2, j-chunk, p(=i within block), jj*128 + dc]
        wd_e = (
            w_down[e]
            .reshape(JT // WDC, WDC, P, DT, P)
            .transpose(3, 0, 2, 1, 4)
            .reshape(DT, JT // WDC, P, WDC * P)
            .astype(bf)
        )
        in_maps.append(
            {
                "xt": xt_e,
                "wgu": np.ascontiguousarray(wgu_e),
                "wd": np.ascontiguousarray(wd_e),
            }
        )
    return in_maps


LAST_RESULTS = None


def kernel(routed_tokens, w_gate_up, w_down):
    global LAST_RESULTS
    if "nc" not in _CACHE:
        _CACHE["nc"] = _build()
    nc = _CACHE["nc"]

    in_maps = _prep_inputs(routed_tokens, w_gate_up, w_down)
    try:
        res = bass_utils.run_bass_kernel_spmd(nc, in_maps, core_ids=list(range(E)))
    except ModuleNotFoundError:
        # BASS_TRACE set but the axon NTFF hook isn't importable here —
        # retry with tracing hard-disabled.
        os.environ["BASS_NEVER_TRACE"] = "1"
        res = bass_utils.run_bass_kernel_spmd(nc, in_maps, core_ids=list(range(E)))
    LAST_RESULTS = res

    out = np.empty((E, T, D), dtype=np.float32)
    for e in range(E):
        out[e] = res.results[e]["outt"].T
    return out.reshape(E * T, D)
